# revision 1
# baseline (speedup 1.0000x reference)
"""Compressed Interaction Network (CIN) kernel for Trainium2, 8 NeuronCores.

Reference computation (per layer l with weights W[F0, Fk, S], bias b[S]):
    z[b,s,d] = relu( sum_{h,k} x0[b,h,d] * xk[b,k,d] * W[h,k,s] + b[s] )
    split_half: xk_next = z[:, :S/2, :], direct_l = z[:, S/2:, :] (last: all)
    out = sum_d concat(direct_0, direct_1, direct_2)    # [B, 64+64+128]

Strategy:
  - Data parallel over batch: each of 8 cores gets B/8 = 256 batches.
  - Per core, work in "transposed" layout [field, bd] with bd = b*16 + d
    (BD = 4096 columns), tiled into 8 column tiles of N=512.
  - Per layer, flatten (h, k) h-major and chunk along the 128-partition dim.
    With Fk=64 a 128-chunk holds exactly 2 h-values, so the xk factor of the
    outer product p[(h,k), bd] = x0[h,bd]*xk[k,bd] is one STATIC tile
    (xkT stacked twice); only the x0 factor needs per-chunk replication:
      * "DVE tiles": replicate via a tiny K=2/3 matmul with a 0/1 matrix
        (PE -> PSUM), multiply on the Vector engine.
      * "GPS tiles": replicate via broadcast-DMA (SBUF), multiply on GpSimd.
    (L0 uses xk = x0 and Fk = 39: chunks of 117 = 3 h-values x 39.)
  - Matmuls accumulate z^T[s, bd] in PSUM over the hk chunks in float32r
    (1 cycle/row at N=512); ScalarE applies bias+relu; DVE reduces over d.
  - Host side transposes/concats per-core [s_cat, b] results to [B, 256].
"""
import numpy as np

import concourse.bass as bass
import concourse.mybir as mybir
from concourse.tile import TileContext
from concourse.bass_utils import run_bass_kernel_spmd

F32 = mybir.dt.float32
F32R = mybir.dt.float32r
MULT = mybir.AluOpType.mult
ADD = mybir.AluOpType.add
RELU = mybir.ActivationFunctionType.Relu
AXX = mybir.AxisListType.X

N_CORES = 8
B, F0, D = 2048, 39, 16
S = 128                    # layer size
BC = B // N_CORES          # 256 batches per core
BD = BC * D                # 4096 columns per core
NT = 512                   # bd-tile width
TILES = BD // NT           # 8
L0_CH, L0_P = 13, 117      # layer-0: 13 chunks of 117 = 3h x 39k
L12_CH = 20                # layers 1/2: 19 full 128-chunks (2h x 64k) + 64
GPS_SEL = (1, 3)           # chunks with c % 5 in GPS_SEL run on GpSimd (40%)
GPS_MOD = 5
GROUP = 4                  # tile-streams interleaved at chunk granularity

MAX_WAITS = 1


def _fix_sync_overflow(nc):
    """This walrus build accepts at most one semaphore wait per instruction;
    Tile can attach several. Hoist extras onto NoOps spliced right before the
    offending instruction on the same engine (same-engine order is
    sequential, so earlier waits are equivalent). Updates stay put."""
    n_new = 0
    for blk in nc.main_func.blocks:
        out = []
        changed = False
        for inst in blk.instructions:
            si = inst.sync_info
            waits = list(si.on_wait) if si is not None else []
            if len(waits) > MAX_WAITS:
                changed = True
                extra, keep = waits[:-MAX_WAITS], waits[-MAX_WAITS:]
                for i in range(0, len(extra), MAX_WAITS):
                    nop = mybir.InstNoOp(name=f"wsplit-{n_new}", ins=[], outs=[])
                    n_new += 1
                    nop.engine = inst.engine
                    nop.sync_info = mybir.SyncInfo(
                        on_wait=extra[i:i + MAX_WAITS], on_update=[])
                    nc.register_instruction(nop, overwrite=True)
                    out.append(nop)
                si.on_wait = keep
            out.append(inst)
        if changed:
            blk.instructions = out
    return n_new


def _build_kernel():
    nc = bass.Bass(trn_type="TRN2")

    x0T = nc.dram_tensor("x0T", [F0, BD], F32, kind="ExternalInput")
    w0 = nc.dram_tensor("w0", [L0_P, L0_CH * S], F32, kind="ExternalInput")
    w1 = nc.dram_tensor("w1", [S, L12_CH * S], F32, kind="ExternalInput")
    w2 = nc.dram_tensor("w2", [S, L12_CH * S], F32, kind="ExternalInput")
    e0 = nc.dram_tensor("e0", [F0, L0_CH * L0_P], F32, kind="ExternalInput")
    e12 = nc.dram_tensor("e12", [F0, L12_CH * S], F32, kind="ExternalInput")
    biases = nc.dram_tensor("biases", [5, S], F32, kind="ExternalInput")
    y = nc.dram_tensor("y", [2 * S, BC], F32, kind="ExternalOutput")

    with TileContext(nc) as tc:
        with tc.tile_pool(name="static", bufs=1) as st, \
             tc.tile_pool(name="p", bufs=10) as pp, \
             tc.tile_pool(name="repd", bufs=8) as rd, \
             tc.tile_pool(name="tmp", bufs=4) as tp, \
             tc.tile_pool(name="zps", bufs=5, space="PSUM") as zp, \
             tc.tile_pool(name="repp", bufs=3, space="PSUM") as rp:

            # ---- static tiles -------------------------------------------
            x0T_s = st.tile([F0, BD], F32R)
            x0rep3_s = st.tile([L0_P, BD], F32)
            xk1_s = st.tile([S, BD], F32)
            xk2_s = st.tile([S, BD], F32)
            w0_s = st.tile([L0_P, L0_CH * S], F32R)
            w1_s = st.tile([S, L12_CH * S], F32R)
            w2_s = st.tile([S, L12_CH * S], F32R)
            e0_s = st.tile([F0, L0_CH * L0_P], F32R)
            e12_s = st.tile([F0, L12_CH * S], F32R)
            bias_s = st.tile([S, 6], F32)   # per-partition bias columns
            o0_s = st.tile([S, BC], F32)
            o1_s = st.tile([S, BC], F32)
            o2_s = st.tile([S, BC], F32)

            nc.sync.dma_start(x0T_s[:, :], x0T[:, :].bitcast(F32R))
            nc.sync.dma_start(e0_s[:, :], e0[:, :].bitcast(F32R))
            for j in range(3):
                nc.sync.dma_start(x0rep3_s[j * F0:(j + 1) * F0, :], x0T[:, :])
            nc.sync.dma_start(w0_s[:, :], w0[:, :].bitcast(F32R))
            nc.sync.dma_start(e12_s[:, :], e12[:, :].bitcast(F32R))
            nc.sync.dma_start(w1_s[:, :], w1[:, :].bitcast(F32R))
            nc.sync.dma_start(w2_s[:, :], w2[:, :].bitcast(F32R))
            # bias columns: [bdup0, bnat0, bdup1, bnat1, bnat2] at cols 0..4
            nc.sync.dma_start(bias_s[:, 0:5],
                              biases[:, :].transpose([1, 0]))

            def layer_gen(t, l, zsrc, xk_next, odst, nch, chp, wt, et,
                          bdup_col, bnat_col):
                """Generator emitting one layer for bd-tile t, yielding after
                each chunk so streams can be interleaved."""
                ts = bass.ts(t, NT)
                zps = zp.tile([S, NT], F32, tag="z")
                for c in range(nch):
                    last = c == nch - 1
                    part = chp if not (l > 0 and last) else 64
                    krep = 3 if l == 0 else (2 if part == chp else 1)
                    gps = (c % GPS_MOD) in GPS_SEL
                    if gps:
                        rep = rd.tile([chp, NT], F32, tag="rep")
                        nrows = F0 if l == 0 else 64
                        h0 = krep * c if l == 0 else 2 * c
                        src = x0T[h0:h0 + krep, ts] \
                            .unsqueeze(1).to_broadcast((krep, nrows, NT))
                        nc.sync.dma_start(rep[:part, :], src)
                        repap = rep[:part, :]
                    else:
                        repps = rp.tile([chp, NT], F32, tag="repps")
                        nc.tensor.matmul(
                            repps[:part, :],
                            et[:, c * chp:c * chp + part],
                            x0T_s[:, ts], start=True, stop=True)
                        repap = repps[:part, :]
                    p = pp.tile([chp, NT], F32R, tag="p")
                    eng = nc.gpsimd if gps else nc.vector
                    eng.tensor_tensor(p[:part, :], zsrc[:part, ts], repap,
                                      op=MULT)
                    nc.tensor.matmul(zps[:, :], wt[:part, bass.ts(c, S)],
                                     p[:part, :], start=(c == 0), stop=last)
                    yield
                # epilogue: bias + relu, xk for next layer, direct reduce
                if xk_next is not None:
                    nc.scalar.activation(xk_next[0:64, ts], zps[0:64, :],
                                         RELU, bias=bias_s[0:64,
                                                           bdup_col:bdup_col + 1])
                    nc.sync.dma_start(xk_next[64:S, ts], xk_next[0:64, ts])
                    tmp = tp.tile([S, NT], F32, tag="tmp")
                    nc.scalar.activation(tmp[64:S, :], zps[64:S, :], RELU,
                                         bias=bias_s[64:S, bnat_col:bnat_col + 1])
                    nc.vector.tensor_reduce(
                        odst[64:S, bass.ts(t, NT // D)],
                        tmp[64:S, :].rearrange("p (b d) -> p b d", d=D),
                        axis=AXX, op=ADD)
                else:
                    tmp = tp.tile([S, NT], F32, tag="tmp")
                    nc.scalar.activation(tmp[:, :], zps[:, :], RELU,
                                         bias=bias_s[:, bnat_col:bnat_col + 1])
                    nc.vector.tensor_reduce(
                        odst[:, bass.ts(t, NT // D)],
                        tmp[:, :].rearrange("p (b d) -> p b d", d=D),
                        axis=AXX, op=ADD)
                yield

            def stream(t):
                yield from layer_gen(t, 0, x0rep3_s, xk1_s, o0_s, L0_CH,
                                     L0_P, w0_s, e0_s, 0, 1)
                yield from layer_gen(t, 1, xk1_s, xk2_s, o1_s, L12_CH,
                                     S, w1_s, e12_s, 2, 3)
                yield from layer_gen(t, 2, xk2_s, None, o2_s, L12_CH,
                                     S, w2_s, e12_s, 4, 4)

            pending = list(range(TILES))
            gens = []
            while gens or pending:
                while len(gens) < GROUP and pending:
                    gens.append(stream(pending.pop(0)))
                for gen in list(gens):
                    try:
                        next(gen)
                    except StopIteration:
                        gens.remove(gen)

            nc.sync.dma_start(y[0:64, :], o0_s[64:S, :])
            nc.sync.dma_start(y[64:S, :], o1_s[64:S, :])
            nc.sync.dma_start(y[S:2 * S, :], o2_s[:, :])

    _fix_sync_overflow(nc)
    return nc


_NC_CACHE = None


def _get_nc():
    global _NC_CACHE
    if _NC_CACHE is None:
        _NC_CACHE = _build_kernel()
    return _NC_CACHE


def _prep_core_inputs(inputs, w_list, b_list, core):
    """Host-side layout prep for one core's batch slice."""
    xs = inputs[core * BC:(core + 1) * BC]          # [BC, F0, D]
    x0t = np.ascontiguousarray(
        xs.transpose(1, 0, 2).reshape(F0, BD)).astype(np.float32)

    w0f, w1f, w2f = w_list
    w0c = np.zeros((L0_P, L0_CH * S), np.float32)
    for c in range(L0_CH):
        for j in range(3):
            for k in range(F0):
                w0c[j * F0 + k, c * S:(c + 1) * S] = w0f[3 * c + j, k]
    wc12 = []
    for wf in (w1f, w2f):
        wc = np.zeros((S, L12_CH * S), np.float32)
        for c in range(L12_CH):
            for j in range(2):
                if 2 * c + j < F0:
                    wc[j * 64:(j + 1) * 64, c * S:(c + 1) * S] = wf[2 * c + j]
        wc12.append(wc)

    e0m = np.zeros((F0, L0_CH * L0_P), np.float32)
    for c in range(L0_CH):
        for m in range(L0_P):
            e0m[3 * c + m // F0, c * L0_P + m] = 1.0
    e12m = np.zeros((F0, L12_CH * S), np.float32)
    for c in range(L12_CH):
        for m in range(S):
            h = 2 * c + m // 64
            if h < F0:
                e12m[h, c * S + m] = 1.0

    b0, b1, b2 = b_list
    biases = np.stack([
        np.concatenate([b0[:64], b0[:64]]), b0,
        np.concatenate([b1[:64], b1[:64]]), b1, b2]).astype(np.float32)

    return {"x0T": x0t, "w0": w0c, "w1": wc12[0], "w2": wc12[1],
            "e0": e0m, "e12": e12m, "biases": biases}


def kernel(inputs, w0, w1, w2, b0, b1, b2, _trace=False):
    inputs = np.asarray(inputs, np.float32)
    w_list = [np.asarray(w, np.float32) for w in (w0, w1, w2)]
    b_list = [np.asarray(b, np.float32) for b in (b0, b1, b2)]

    nc = _get_nc()
    in_maps = [_prep_core_inputs(inputs, w_list, b_list, core)
               for core in range(N_CORES)]
    res = run_bass_kernel_spmd(nc, in_maps, core_ids=list(range(N_CORES)),
                               trace=_trace)
    outs = []
    for core in range(N_CORES):
        yc = res.results[core]["y"]          # [256 s_cat, 256 b]
        outs.append(np.ascontiguousarray(yc.T))
    full = np.concatenate(outs, axis=0)       # [2048, 256]
    if _trace:
        return full, res
    return full



# revision 5
# speedup vs baseline: 3.5164x; 3.5164x over previous
"""Compressed Interaction Network (CIN) kernel for Trainium2, 8 NeuronCores.

Reference computation (per layer l with weights W[F0, Fk, S], bias b[S]):
    z[b,s,d] = relu( sum_{h,k} x0[b,h,d] * xk[b,k,d] * W[h,k,s] + b[s] )
    split_half: xk_next = z[:, :S/2, :], direct_l = z[:, S/2:, :] (last: all)
    out = sum_d concat(direct_0, direct_1, direct_2)    # [B, 64+64+128]

Strategy (v2 — fp16, DRAM-streamed replication):
  - Data parallel over batch: each of 8 cores gets B/8 = 256 batches,
    working in transposed layout [field, bd], bd = b*16 + d (BD = 4096).
  - Per layer, flatten (h, k) h-major into 128-row chunks. The moving
    matmul operand p[(h,k), bd] = x0[h,bd] * xk[k,bd] is built on DVE /
    GpSimd as (replicated x0) * (stacked xk), all in fp16 (DVE 2x mode).
  - The replicated-x0 factor is precomputed on the HOST and streamed from
    DRAM in fp16 with fat (16KB/partition) descriptors — no on-chip
    broadcast DMAs, no replication matmuls. Layers 1 and 2 share the same
    replication pattern, so their rep tiles are loaded once and held in
    SBUF across both layers.
  - bd is split into 2 half-BD groups of 2048 cols (4 bd-tiles of 512).
    Per group: L0 (13 chunks x 117 rows) -> L1 -> L2 (20 chunks x 128).
    z accumulates in PSUM fp32 (8 banks = 2 groups x 4 tiles); fp16
    matmuls run at 1 cycle/row at any PE p-state.
  - Epilogue: ScalarE relu+bias -> fp16 (xk halves + direct tmp), one fat
    SBUF copy duplicates the stacked xk half, DVE reduces over d.
"""
import numpy as np

import concourse.bass as bass
import concourse.mybir as mybir
from concourse.tile import TileContext
from concourse.bass_utils import run_bass_kernel_spmd

F32 = mybir.dt.float32
F16 = mybir.dt.float16
MULT = mybir.AluOpType.mult
ADD = mybir.AluOpType.add
RELU = mybir.ActivationFunctionType.Relu
AXX = mybir.AxisListType.X

N_CORES = 8
B, F0, D = 2048, 39, 16
S = 128                     # layer size
BC = B // N_CORES           # 256 batches per core
BD = BC * D                 # 4096 columns per core
NGRP = 2                    # half-BD groups
GW = BD // NGRP             # 2048 cols per group
NT = 512                    # bd-tile width (PSUM bank)
TPG = GW // NT              # 4 tiles per group
L0_CH, L0_P = 13, 117       # layer-0: 13 chunks of 117 = 3h x 39k
L12_CH = 20                 # layers 1/2: 19 full 128-chunks + one 64-chunk
BND = 4                     # rep chunks per DMA bundle
GPS_L0 = (5, 11)            # chunk ids multiplied on GpSimd
GPS_L12 = (4, 9, 14, 19)

MAX_WAITS = 1


def _fix_sync_overflow(nc):
    """This walrus build accepts at most one semaphore wait per instruction;
    Tile can attach several. Hoist extras onto NoOps spliced right before the
    offending instruction on the same engine (same-engine order is
    sequential, so earlier waits are equivalent). Updates stay put."""
    n_new = 0
    for blk in nc.main_func.blocks:
        out = []
        changed = False
        for inst in blk.instructions:
            si = inst.sync_info
            waits = list(si.on_wait) if si is not None else []
            if len(waits) > MAX_WAITS:
                changed = True
                extra, keep = waits[:-MAX_WAITS], waits[-MAX_WAITS:]
                for i in range(0, len(extra), MAX_WAITS):
                    nop = mybir.InstNoOp(name=f"wsplit-{n_new}", ins=[], outs=[])
                    n_new += 1
                    nop.engine = inst.engine
                    nop.sync_info = mybir.SyncInfo(
                        on_wait=extra[i:i + MAX_WAITS], on_update=[])
                    nc.register_instruction(nop, overwrite=True)
                    out.append(nop)
                si.on_wait = keep
            out.append(inst)
        if changed:
            blk.instructions = out
    return n_new


def _build_kernel():
    nc = bass.Bass(trn_type="TRN2")

    rep0 = nc.dram_tensor("rep0", [L0_P, NGRP, L0_CH, GW], F16,
                          kind="ExternalInput")
    rep12 = nc.dram_tensor("rep12", [S, NGRP, L12_CH, GW], F16,
                           kind="ExternalInput")
    x0st3 = nc.dram_tensor("x0st3", [L0_P, BD], F16, kind="ExternalInput")
    w0 = nc.dram_tensor("w0", [L0_P, L0_CH * S], F16, kind="ExternalInput")
    w1 = nc.dram_tensor("w1", [S, L12_CH * S], F16, kind="ExternalInput")
    w2 = nc.dram_tensor("w2", [S, L12_CH * S], F16, kind="ExternalInput")
    biases = nc.dram_tensor("biases", [S, 4], F32, kind="ExternalInput")
    y = nc.dram_tensor("y", [2 * S, BC], F32, kind="ExternalOutput")

    with TileContext(nc) as tc:
        with tc.tile_pool(name="static", bufs=1) as st, \
             tc.tile_pool(name="rep", bufs=7) as rp, \
             tc.tile_pool(name="p", bufs=4) as pp, \
             tc.tile_pool(name="tmp", bufs=3) as tp, \
             tc.tile_pool(name="zps", bufs=8, space="PSUM") as zp:

            # ---- static tiles -------------------------------------------
            x0st3_s = st.tile([L0_P, BD], F16)
            xk1_s = st.tile([S, BD], F16)
            xk2_s = st.tile([S, BD], F16)
            w0_s = st.tile([L0_P, L0_CH * S], F16)
            w1_s = st.tile([S, L12_CH * S], F16)
            w2_s = st.tile([S, L12_CH * S], F16)
            bias_s = st.tile([S, 4], F32)
            o0_s = st.tile([S, BC], F32)
            o1_s = st.tile([S, BC], F32)
            o2_s = st.tile([S, BC], F32)

            nc.sync.dma_start(x0st3_s[:, :], x0st3[:, :])
            nc.sync.dma_start(w0_s[:, :], w0[:, :])
            nc.scalar.dma_start(w1_s[:, :], w1[:, :])
            nc.scalar.dma_start(w2_s[:, :], w2[:, :])
            nc.sync.dma_start(bias_s[:, :], biases[:, :])

            dma_eng = [nc.sync, nc.scalar]

            def layer_gen(g, l, rep_hold, xk_next, odst, bias_col):
                """Emit one layer for group g, yielding after each chunk."""
                gof = g * GW
                nch = L0_CH if l == 0 else L12_CH
                part_full = L0_P if l == 0 else S
                in0 = x0st3_s if l == 0 else (xk1_s if l == 1 else xk2_s)
                wt = w0_s if l == 0 else (w1_s if l == 1 else w2_s)
                gps_set = GPS_L0 if l == 0 else GPS_L12
                zs = [zp.tile([S, NT], F32, tag="z", name=f"z{g}{l}{t}")
                      for t in range(TPG)]
                bundle = None
                for c in range(nch):
                    part = 64 if (l > 0 and c == nch - 1) else part_full
                    bi, ci = divmod(c, BND)
                    if ci == 0:
                        if l == 2:
                            bundle = rep_hold[bi]
                        else:
                            nb = min(BND, nch - c)
                            bundle = rp.tile([S, BND * GW], F16, tag="rep")
                            src = rep0 if l == 0 else rep12
                            eng = dma_eng[(g + bi) % 2]
                            eng.dma_start(
                                bundle[:part_full, 0:nb * GW],
                                src[0:part_full, g, c:c + nb, :])
                            if l == 1:
                                rep_hold.append(bundle)
                    rep_ap = bundle[:part, ci * GW:(ci + 1) * GW]
                    p = pp.tile([S, GW], F16, tag="p")
                    eng = nc.gpsimd if c in gps_set else nc.vector
                    eng.tensor_tensor(p[:part, :], in0[:part, gof:gof + GW],
                                      rep_ap, op=MULT)
                    for t in range(TPG):
                        nc.tensor.matmul(
                            zs[t][:, :], wt[:part, bass.ts(c, S)],
                            p[:part, bass.ts(t, NT)],
                            start=(c == 0), stop=(c == nch - 1))
                    yield
                # epilogue: bias + relu -> fp16; xk halves; direct reduce
                for t in range(TPG):
                    ts = bass.ts(g * TPG + t, NT)
                    ocol = bass.ts(g * TPG + t, NT // D)
                    bias_ap = bias_s[:, bias_col:bias_col + 1]
                    if xk_next is not None:
                        nc.scalar.activation(
                            xk_next[0:64, ts], zs[t][0:64, :], RELU,
                            bias=bias_s[0:64, bias_col:bias_col + 1])
                        tmp = tp.tile([S, NT], F16, tag="tmp")
                        nc.scalar.activation(
                            tmp[64:S, :], zs[t][64:S, :], RELU,
                            bias=bias_s[64:S, bias_col:bias_col + 1])
                        nc.vector.tensor_reduce(
                            odst[64:S, ocol],
                            tmp[64:S, :].rearrange("p (b d) -> p b d", d=D),
                            axis=AXX, op=ADD)
                    else:
                        tmp = tp.tile([S, NT], F16, tag="tmp")
                        nc.scalar.activation(tmp[:, :], zs[t][:, :], RELU,
                                             bias=bias_ap)
                        nc.vector.tensor_reduce(
                            odst[:, ocol],
                            tmp[:, :].rearrange("p (b d) -> p b d", d=D),
                            axis=AXX, op=ADD)
                if xk_next is not None:
                    # duplicate the stacked xk half with one fat SBUF copy
                    dma_eng[g % 2].dma_start(
                        xk_next[64:S, gof:gof + GW],
                        xk_next[0:64, gof:gof + GW])
                yield

            def run(*gens):
                gens = list(gens)
                while gens:
                    for gen in list(gens):
                        try:
                            next(gen)
                        except StopIteration:
                            gens.remove(gen)

            # Sequential phases with one interleave zone (L2(A) || L1(B)).
            # All tile-pool ring-slot reuses point backward in trace order,
            # which keeps the per-engine in-order queues deadlock-free.
            repA, repB = [], []
            run(layer_gen(0, 0, None, xk1_s, o0_s, 0))
            run(layer_gen(1, 0, None, xk1_s, o0_s, 0))
            run(layer_gen(0, 1, repA, xk2_s, o1_s, 1))
            run(layer_gen(0, 2, repA, None, o2_s, 2),
                layer_gen(1, 1, repB, xk2_s, o1_s, 1))
            run(layer_gen(1, 2, repB, None, o2_s, 2))

            nc.sync.dma_start(y[0:64, :], o0_s[64:S, :])
            nc.scalar.dma_start(y[64:S, :], o1_s[64:S, :])
            nc.sync.dma_start(y[S:2 * S, :], o2_s[:, :])

    _fix_sync_overflow(nc)
    return nc


_NC_CACHE = None


def _get_nc():
    global _NC_CACHE
    if _NC_CACHE is None:
        _NC_CACHE = _build_kernel()
    return _NC_CACHE


def _prep_core_inputs(x16, w_list, b_list, core):
    """Host-side layout prep for one core's batch slice. x16: [F0, B*D] f16
    full-batch transposed input."""
    x0t = x16[:, core * BD:(core + 1) * BD]          # [39, 4096] f16

    # rep0[(j*39+k), g, c, col] = x0t[3c+j, g*2048+col]
    a = np.ascontiguousarray(x0t).reshape(L0_CH, 3, NGRP, GW)
    rep0 = np.broadcast_to(a[:, :, None, :, :],
                           (L0_CH, 3, F0, NGRP, GW))
    rep0 = np.ascontiguousarray(rep0.transpose(1, 2, 3, 0, 4)) \
        .reshape(L0_P, NGRP, L0_CH, GW)

    # rep12[(j*64+k), g, c, col] = x0t[2c+j, g*2048+col]  (h=39 row zero)
    xp = np.concatenate([x0t, np.zeros((1, BD), np.float16)], axis=0)
    a = xp.reshape(L12_CH, 2, NGRP, GW)
    rep12 = np.broadcast_to(a[:, :, None, :, :],
                            (L12_CH, 2, 64, NGRP, GW))
    rep12 = np.ascontiguousarray(rep12.transpose(1, 2, 3, 0, 4)) \
        .reshape(S, NGRP, L12_CH, GW)

    x0st3 = np.ascontiguousarray(np.tile(x0t, (3, 1)))   # [117, 4096]

    return {"rep0": rep0, "rep12": rep12, "x0st3": x0st3,
            "w0": w_list[0], "w1": w_list[1], "w2": w_list[2],
            "biases": b_list}


def kernel(inputs, w0, w1, w2, b0, b1, b2, _trace=False):
    inputs = np.asarray(inputs, np.float32)
    x16 = np.ascontiguousarray(
        inputs.transpose(1, 0, 2).reshape(F0, B * D)).astype(np.float16)

    w0f = np.asarray(w0, np.float32)
    w1f = np.asarray(w1, np.float32)
    w2f = np.asarray(w2, np.float32)
    # w0c[(j*39+k), c*128:+128] = w0f[3c+j, k, :]
    w0c = np.ascontiguousarray(
        w0f.reshape(L0_CH, 3, F0, S).transpose(1, 2, 0, 3)
        .reshape(L0_P, L0_CH * S)).astype(np.float16)
    wc12 = []
    for wf in (w1f, w2f):
        wp = np.concatenate([wf, np.zeros((1, 64, S), np.float32)], axis=0)
        wc = np.ascontiguousarray(
            wp.reshape(L12_CH, 2, 64, S).transpose(1, 2, 0, 3)
            .reshape(S, L12_CH * S)).astype(np.float16)
        wc12.append(wc)
    w_list = [w0c, wc12[0], wc12[1]]

    bmat = np.zeros((S, 4), np.float32)
    bmat[:, 0] = np.asarray(b0, np.float32)
    bmat[:, 1] = np.asarray(b1, np.float32)
    bmat[:, 2] = np.asarray(b2, np.float32)

    nc = _get_nc()
    in_maps = [_prep_core_inputs(x16, w_list, bmat, core)
               for core in range(N_CORES)]
    res = run_bass_kernel_spmd(nc, in_maps, core_ids=list(range(N_CORES)),
                               trace=_trace)
    outs = []
    for core in range(N_CORES):
        yc = res.results[core]["y"]          # [256 s_cat, 256 b]
        outs.append(np.ascontiguousarray(yc.T))
    full = np.concatenate(outs, axis=0)       # [2048, 256]
    if _trace:
        return full, res
    return full
